# revision 27
# baseline (speedup 1.0000x reference)
"""Self-contained Trainium2 Bass kernel for nn_CustomRNN (linear Elman RNN).

Math: h_t = h_{t-1} @ W2^T + x_t * W1[:,0]; y_t = W3 @ h_t^T, output [O, T, B].
With A = W2^T this collapses to a chunked linear scan (chunk C=128):
  y_{jC+i}^T = h_{jC} @ (A^{i+1} W3^T) + sum_{s<=i} x[:, jC+s] * v_{i-s}
  h_{(j+1)C} = h_{jC} @ A^C + X_j @ Urev,
with U[k,:] = w1 A^k, v_k = U[k,:] @ W3^T, all built on device from matrix
squarings A^2..A^128 (the first squaring uses a 3-pass hi/lo split for accuracy).

Sharding: data-parallel over batch B: each of 8 cores handles 32 batch rows.
All matmuls run in fp32r (full-rate). Host only rounds/transposes inputs and
reassembles the output.
"""
import numpy as np
from contextlib import ExitStack

import concourse.bass as bass
import concourse.tile as tile
from concourse import bacc, mybir
from concourse.bass_utils import run_bass_kernel_spmd

F32 = mybir.dt.float32
F32R = mybir.dt.float32r

B, T, H, O = 256, 4096, 1024, 32
C = 128            # chunk size
NCH = T // C       # 32 chunks
HK = H // 128      # 8 h-tiles
BL = B // 8        # 32 batch rows per core
NCORES = 8


def _r32r(x):
    """Round fp32 -> fp32r (11 explicit mantissa bits, RNE) — matches HW."""
    x = np.ascontiguousarray(x, np.float32)
    b = x.view(np.uint32).copy()
    low = b & np.uint32(0xFFF)
    b &= np.uint32(0xFFFFF000)
    up = (low > 0x800) | ((low == 0x800) & (((b >> 12) & 1) == 1))
    b += np.where(up, np.uint32(0x1000), np.uint32(0)).astype(np.uint32)
    out = b.view(np.float32)
    bad = ~np.isfinite(x)
    if bad.any():
        out = np.where(bad, x, out)
    return np.ascontiguousarray(out)


def _bsl(bigtile, kp, c0, w):
    """Slice of a [1024,1024] matrix stored as SBUF bigtile [128, 8192]:
    rows kp*128..+128, cols c0..c0+w."""
    return bigtile[:, kp * 1024 + c0: kp * 1024 + c0 + w]


def build_nc(debug=False, ndev=NCORES):
    _ctr = [0]

    def _nm(base):
        _ctr[0] += 1
        return f"{base}{_ctr[0]}"
    nc = bacc.Bacc("TRN2", target_bir_lowering=False, debug=False,
                   num_devices=ndev)

    # ---- external I/O (per core) ----
    w2t_hi = nc.dram_tensor("w2t_hi", [H, H], F32R, kind="ExternalInput").ap()
    w2t_lo = nc.dram_tensor("w2t_lo", [H, H], F32R, kind="ExternalInput").ap()
    w2_hi = nc.dram_tensor("w2_hi", [H, H], F32R, kind="ExternalInput").ap()
    w2_lo = nc.dram_tensor("w2_lo", [H, H], F32R, kind="ExternalInput").ap()
    w3t_in = nc.dram_tensor("w3t", [H, O], F32R, kind="ExternalInput").ap()
    w1_in = nc.dram_tensor("w1r", [H, 1], F32R, kind="ExternalInput").ap()
    xt_in = nc.dram_tensor("xT", [T, BL], F32R, kind="ExternalInput").ap()
    xrevt_in = nc.dram_tensor("xrevT", [T, BL], F32R, kind="ExternalInput").ap()
    ident_in = nc.dram_tensor("ident", [128, 128], F32R, kind="ExternalInput").ap()
    zeros_in = nc.dram_tensor("zeros", [128, O], F32R, kind="ExternalInput").ap()
    y_out = nc.dram_tensor("y_out", [H, T], F32, kind="ExternalOutput").ap()

    # ---- DRAM scratch ----
    pdram = nc.dram_tensor("pdram", [H, C * O], F32R).ap()
    vpad = nc.dram_tensor("vpad", [(C - 1) * O + C * O], F32R).ap()
    statesd = nc.dram_tensor("statesd", [H, NCH * BL], F32R).ap()

    dbg = {}
    if debug:
        dbg["dbg_s128"] = nc.dram_tensor("dbg_s128", [H, H], F32R,
                                         kind="ExternalOutput").ap()
        dbg["dbg_states"] = nc.dram_tensor("dbg_states", [H, NCH * BL], F32R,
                                           kind="ExternalOutput").ap()
        dbg["dbg_vpad"] = nc.dram_tensor("dbg_vpad", [(C - 1) * O + C * O], F32R,
                                         kind="ExternalOutput").ap()
        dbg["dbg_p"] = nc.dram_tensor("dbg_p", [H, C * O], F32R,
                                      kind="ExternalOutput").ap()

    with tile.TileContext(nc) as tc, ExitStack() as gctx:
        # ---------- global (kernel-lifetime) pools ----------
        gsb = gctx.enter_context(tc.tile_pool(name="gsb", bufs=1))
        stg_r = gctx.enter_context(tc.tile_pool(name="stg_r", bufs=2))
        stg_f = gctx.enter_context(tc.tile_pool(name="stg_f", bufs=2))
        xw = gctx.enter_context(tc.tile_pool(name="xw", bufs=3))

        ut_t = gsb.tile([128, H], F32R, tag="ut", name="ut_t")       # Ut[h,k]: [p, kp*128+k]
        u_t = gsb.tile([128, H], F32R, tag="u", name="u_t")         # U[k,h]:  [p, kp*128+p']->U[k=p, kp*128+p']
        w3t_t = gsb.tile([128, HK * O], F32R, tag="w3t", name="w3t_t")  # W3T[h,o]: [p, kp*32+o]
        ident_t = gsb.tile([128, 128], F32R, tag="ident", name="ident_t")

        nc.sync.dma_start(ident_t[:], ident_in[:])
        for kp in range(HK):
            nc.sync.dma_start(w3t_t[:, kp * O:(kp + 1) * O],
                              w3t_in[kp * 128:(kp + 1) * 128, :])
            # pre-zero the 32-wide pad window read by the padded U-level
            # matmuls, then drop w1 into column 0
            nc.sync.dma_start(ut_t[:, kp * 128: kp * 128 + O], zeros_in[:])
            nc.sync.dma_start(ut_t[:, kp * 128: kp * 128 + 1],
                              w1_in[kp * 128:(kp + 1) * 128, :])

        # The A^128 chain operator lives from the last squaring (which writes
        # it directly, no DRAM round-trip) through the chain.
        octx = gctx.enter_context(ExitStack())
        s128p = octx.enter_context(tc.tile_pool(name="s128", bufs=1))
        s128_t = s128p.tile([128, HK * 1024], F32R, tag="s128", name=_nm("s128t"))

        # ================= Phase SQ: squarings + U doubling =================
        with ExitStack() as ctx:
            big = ctx.enter_context(tc.tile_pool(name="big", bufs=4))
            los = ctx.enter_context(tc.tile_pool(name="los", bufs=2))
            prq = ctx.enter_context(tc.tile_pool(name="prq", bufs=8))
            sqp = ctx.enter_context(tc.tile_pool(name="sqp", bufs=5, space="PSUM"))
            trp = ctx.enter_context(tc.tile_pool(name="trp", bufs=2, space="PSUM"))
            uvp = ctx.enter_context(tc.tile_pool(name="uvp", bufs=1, space="PSUM"))

            def bigtile():
                return big.tile([128, HK * 1024], F32R, tag="bigmat", name=_nm("bm"))

            def load_big(dst, src_dram):
                for kp in range(HK):
                    nc.sync.dma_start(dst[:, kp * 1024:(kp + 1) * 1024],
                                      src_dram[kp * 128:(kp + 1) * 128, :])

            def store_big(dst_dram, src):
                for kp in range(HK):
                    nc.sync.dma_start(dst_dram[kp * 128:(kp + 1) * 128, :],
                                      src[:, kp * 1024:(kp + 1) * 1024])

            def emit_transpose(dst, src):
                """dst = src^T (both [1024,1024] bigtiles)."""
                for kp in range(HK):
                    for mt in range(HK):
                        p = trp.tile([128, 128], F32R, tag="tr", name=_nm("tr"))
                        nc.tensor.transpose(p[:], _bsl(src, kp, mt * 128, 128),
                                            ident_t[:])
                        nc.vector.tensor_copy(_bsl(dst, mt, kp * 128, 128), p[:])

            def emit_mm(passes, dst):
                """dst = sum over (LT,R) passes of LT^T@R; all bigtiles."""
                np_ = len(passes)
                for mt in range(HK):
                    for nci in range(2):
                        p = sqp.tile([128, 512], F32, tag="sq", name=_nm("sq"))
                        for pi, (LT, R) in enumerate(passes):
                            for kp in range(HK):
                                nc.tensor.matmul(
                                    p[:],
                                    _bsl(LT, kp, mt * 128, 128),
                                    _bsl(R, kp, nci * 512, 512),
                                    start=(pi == 0 and kp == 0),
                                    stop=(pi == np_ - 1 and kp == HK - 1))
                        nc.vector.tensor_copy(
                            _bsl(dst, mt, nci * 512, 512), p[:])

            def emit_u_level(m, sh_big):
                """Ut cols [m,2m) = (A^m)^T-propagated: lhsT=S_m resident.
                N padded to >=32 to satisfy fp32r matmul ISA restrictions;
                the garbage columns beyond m are never copied out."""
                mw = max(m, 32)
                for mt in range(HK):
                    p = uvp.tile([128, 64], F32, tag="u", name=_nm("up"))
                    for kp in range(HK):
                        nc.tensor.matmul(
                            p[:, :mw],
                            _bsl(sh_big, kp, mt * 128, 128),
                            ut_t[:, kp * 128: kp * 128 + mw],
                            start=(kp == 0), stop=(kp == HK - 1))
                    nc.vector.tensor_copy(
                        ut_t[:, mt * 128 + m: mt * 128 + 2 * m], p[:, :m])

            def pblock_res(lt_big, m):
                """P cols [m*O, 2m*O) = A^m @ P[0:m*O), lhsT = resident
                (A^m)^T bigtile; PSUMs shared with the squaring pool so the
                scheduler interleaves P work into squaring gaps."""
                ncols = O if m == 0 else m * O
                for c0 in range(0, ncols, 512):
                    w = min(512, ncols - c0)
                    rts = []
                    for kp in range(HK):
                        if m == 0:
                            rts.append(w3t_t[:, kp * O:(kp + 1) * O])
                        else:
                            rt = prq.tile([128, 512], F32R, tag="pr", name=_nm("pr"))
                            nc.sync.dma_start(
                                rt[:, :w],
                                pdram[kp * 128:(kp + 1) * 128, c0:c0 + w])
                            rts.append(rt[:, :w])
                    for wv in range(4):
                        psums = [sqp.tile([128, 512], F32, tag="sq", name=_nm("pq"))
                                 for _ in range(2)]
                        for kp in range(HK):
                            for mi, mt in enumerate(range(wv * 2, wv * 2 + 2)):
                                nc.tensor.matmul(
                                    psums[mi][:, :w],
                                    _bsl(lt_big, kp, mt * 128, 128), rts[kp],
                                    start=(kp == 0), stop=(kp == HK - 1))
                        for mi, mt in enumerate(range(wv * 2, wv * 2 + 2)):
                            st = stg_r.tile([128, 512], F32R, tag="pst", name=_nm("pst"))
                            nc.vector.tensor_copy(st[:, :w], psums[mi][:, :w])
                            nc.sync.dma_start(
                                pdram[mt * 128:(mt + 1) * 128,
                                      (0 if m == 0 else m * O) + c0:
                                      (0 if m == 0 else m * O) + c0 + w],
                                st[:, :w])

            # ---- level 1 (split: S2 = AhAh + AhAl + AlAh), streamed lows ----
            a_hi = bigtile()      # W2T_hi  (= S_1 head, rhs role)
            at_hi = bigtile()     # W2_hi   (= ST_1 head, lhsT role)
            load_big(a_hi, w2t_hi)
            load_big(at_hi, w2_hi)

            emit_u_level(1, a_hi)   # U level m=1 uses S_1 = A

            sh2 = bigtile()
            for nci in range(2):
                for wv in range(2):
                    psums = [sqp.tile([128, 512], F32, tag="sq", name=_nm("sq"))
                             for _ in range(4)]
                    # pass 0: Ah^T @ Ah ; pass 1: Ah^T @ Al ; pass 2: Al^T @ Ah
                    for pi in range(3):
                        for kp in range(HK):
                            if pi == 1:
                                rlo = los.tile([128, 512], F32R, tag="rlo", name=_nm("rlo"))
                                nc.sync.dma_start(
                                    rlo[:], w2t_lo[kp * 128:(kp + 1) * 128,
                                                   nci * 512: nci * 512 + 512])
                            if pi == 2:
                                llo = los.tile([128, 1024], F32R, tag="llo", name=_nm("llo"))
                                nc.sync.dma_start(
                                    llo[:], w2_lo[kp * 128:(kp + 1) * 128, :])
                            for mi, mt in enumerate(range(wv * 4, wv * 4 + 4)):
                                lt = (_bsl(at_hi, kp, mt * 128, 128) if pi < 2
                                      else llo[:, mt * 128:(mt + 1) * 128])
                                rh = (_bsl(a_hi, kp, nci * 512, 512) if pi != 1
                                      else rlo[:])
                                nc.tensor.matmul(
                                    psums[mi][:], lt, rh,
                                    start=(pi == 0 and kp == 0),
                                    stop=(pi == 2 and kp == HK - 1))
                    for mi, mt in enumerate(range(wv * 4, wv * 4 + 4)):
                        nc.vector.tensor_copy(
                            _bsl(sh2, mt, nci * 512, 512), psums[mi][:])

            pblock_res(at_hi, 0)              # P_0 = A @ W3T
            pblock_res(at_hi, 1)
            emit_u_level(2, sh2)
            sh2t = bigtile()
            emit_transpose(sh2t, sh2)
            pblock_res(sh2t, 2)

            # ---- level 2 (plain fp32r): S4 = Sh2 @ Sh2 ----
            sh4 = bigtile()
            emit_mm([(sh2t, sh2)], sh4)
            emit_u_level(4, sh4)
            sh4t = bigtile()
            emit_transpose(sh4t, sh4)
            pblock_res(sh4t, 4)

            # ---- levels 3..7 (plain fp32r) ----
            cur, curt, m = sh4, sh4t, 4
            for lvl in range(3, 7):
                nxt = bigtile()
                emit_mm([(curt, cur)], nxt)
                m *= 2
                emit_u_level(m, nxt)
                nxtt = bigtile()
                emit_transpose(nxtt, nxt)
                pblock_res(nxtt, m)
                cur, curt = nxt, nxtt
            emit_mm([(curt, cur)], s128_t)   # A^128 -> chain operator tile

            # ---- v = Ut^T @ W3T : [C, O], then vpad in DRAM ----
            vp = uvp.tile([128, 64], F32, tag="u", name=_nm("vp"))[:, :O]
            for kp in range(HK):
                nc.tensor.matmul(vp[:], ut_t[:, kp * 128:(kp + 1) * 128],
                                 w3t_t[:, kp * O:(kp + 1) * O],
                                 start=(kp == 0), stop=(kp == HK - 1))
            v_sb = stg_r.tile([128, O], F32R, tag="vsb", name=_nm("vsb"))
            nc.vector.tensor_copy(v_sb[:], vp[:])
            z_sb = stg_r.tile([128, O], F32R, tag="zsb", name=_nm("zsb"))
            nc.vector.tensor_sub(z_sb[:], v_sb[:], v_sb[:])
            nc.sync.dma_start(
                vpad[0:(C - 1) * O].rearrange("(p o) -> p o", o=O),
                z_sb[0:C - 1, :])
            nc.sync.dma_start(
                vpad[(C - 1) * O:].rearrange("(p o) -> p o", o=O), v_sb[:])

            # ---- U (transposed Ut) for B_j's ----
            for kp in range(HK):
                p = trp.tile([128, 128], F32R, tag="tr", name=_nm("tr"))
                nc.tensor.transpose(p[:], ut_t[:, kp * 128:(kp + 1) * 128],
                                    ident_t[:])
                nc.vector.tensor_copy(u_t[:, kp * 128:(kp + 1) * 128], p[:])

        # ===== Phase P+chain (interleaved): P doubling is DMA-heavy, the
        # sequential chunk-state chain is PE-heavy; they are independent, so
        # emitting them in one pool scope lets Tile overlap them. ===========
        # chunk-state tiles span the chain AND the output phase
        stp = octx.enter_context(tc.tile_pool(name="stp", bufs=1))
        stall = [stp.tile([128, NCH * BL], F32R, tag=f"st{k}", name=_nm("sta"))
                 for k in range(HK)]

        with ExitStack() as ctx:
            chp = ctx.enter_context(tc.tile_pool(name="chp", bufs=4, space="PSUM"))
            toep = ctx.enter_context(tc.tile_pool(name="toep", bufs=1))
            xgp = ctx.enter_context(tc.tile_pool(name="xgp", bufs=1))
            prs = ctx.enter_context(tc.tile_pool(name="prs", bufs=8))
            s4p = ctx.enter_context(tc.tile_pool(name="s4p", bufs=4, space="PSUM"))

            # ---- chain emitted FIRST: its serial dependency gets scheduler
            # priority; P-doubling below fills the PE/DMA gaps. The bt_j
            # input contribution is folded into the chain PSUM group as a
            # 9th accumulating matmul (no separate bt phase, no DVE add). ----
            for k in range(HK):
                nc.sync.dma_start(stall[k][:, 0:BL], zeros_in[:])
            for j in range(NCH - 1):
                xr = xw.tile([128, BL], F32R, tag="xr", name=_nm("xr"))
                nc.sync.dma_start(xr[:], xrevt_in[T - (j + 1) * C:
                                                  T - j * C, :])
                for mt in range(HK):
                    p = chp.tile([128, BL], F32, tag="chp", name=_nm("chp"))
                    for kp in range(HK):
                        nc.tensor.matmul(
                            p[:], _bsl(s128_t, kp, mt * 128, 128),
                            stall[kp][:, j * BL:(j + 1) * BL],
                            start=(kp == 0), stop=False)
                    nc.tensor.matmul(p[:], u_t[:, mt * 128:(mt + 1) * 128],
                                     xr[:], start=False, stop=True)
                    nc.vector.tensor_copy(
                        stall[mt][:, (j + 1) * BL:(j + 2) * BL], p[:])
            if debug:
                for k in range(HK):
                    nc.sync.dma_start(statesd[k * 128:(k + 1) * 128, :],
                                      stall[k][:])


            # ---- output phase, same scope: a step4 M-tile t only needs
            # chain states 4t..4t+3 and its P columns, so the scheduler can
            # interleave these tiles into chain/P gaps ----
            stback = stall
            # Flipped-contraction layout: partition index s' = C-1-s, so the
            # Toeplitz load uses positive strides and the x operand comes from
            # the already-reversed xrevT input.
            toe2 = toep.tile([128, C * O], F32R, tag="toe", name=_nm("toe"))
            toe_src = bass.AP(vpad.tensor, 0, [[O, 128], [1, C * O]])
            nc.sync.dma_start(toe2[:], toe_src)
            xtg = [xgp.tile([128, 128], F32R, tag=f"xg{t}", name=_nm("xg")) for t in range(HK)]
            for t in range(HK):
                for q in range(4):
                    j = 4 * t + q
                    nc.sync.dma_start(xtg[t][:, q * BL:(q + 1) * BL],
                                      xrevt_in[T - (j + 1) * C: T - j * C, :])

            for n in range(HK):
                prhs = [prs.tile([128, 512], F32R, tag="prhs", name=_nm("prh"))
                        for _ in range(HK)]
                for kp in range(HK):
                    nc.sync.dma_start(
                        prhs[kp][:],
                        pdram[kp * 128:(kp + 1) * 128, n * 512:(n + 1) * 512])
                for t in range(HK):
                    p = s4p.tile([128, 512], F32, tag="s4", name=_nm("s4"))
                    for kp in range(HK):
                        nc.tensor.matmul(
                            p[:], stback[kp][:, t * 128:(t + 1) * 128],
                            prhs[kp][:], start=(kp == 0), stop=False)
                    nc.tensor.matmul(p[:], xtg[t][:],
                                     toe2[:, n * 512:(n + 1) * 512],
                                     start=False, stop=True)
                    st = stg_f.tile([128, 512], F32, tag="yst", name=_nm("yst"))
                    nc.vector.tensor_copy(st[:], p[:])
                    nc.sync.dma_start(
                        y_out[t * 128:(t + 1) * 128, n * 512:(n + 1) * 512],
                        st[:])

        if debug:
            for kp in range(HK):
                nc.sync.dma_start(dbg["dbg_s128"][kp * 128:(kp + 1) * 128, :],
                                  s128_t[:, kp * 1024:(kp + 1) * 1024])
            nc.sync.dma_start(dbg["dbg_states"][:, :], statesd[:, :])
            nc.sync.dma_start(dbg["dbg_vpad"][:], vpad[:])
            nc.sync.dma_start(dbg["dbg_p"][:, :], pdram[:, :])

    nc.compile()
    return nc


_NC_CACHE = {}


def _get_nc(debug=False):
    if debug not in _NC_CACHE:
        _NC_CACHE[debug] = build_nc(debug)
    return _NC_CACHE[debug]


def make_in_maps(x, W1, W2, W3):
    A = np.ascontiguousarray(W2.T.astype(np.float32))
    a_hi = _r32r(A)
    a_lo = _r32r(A - a_hi)
    common = {
        "w2t_hi": a_hi,
        "w2t_lo": a_lo,
        "w2_hi": np.ascontiguousarray(a_hi.T),
        "w2_lo": np.ascontiguousarray(a_lo.T),
        "w3t": _r32r(W3.T),
        "w1r": _r32r(W1.reshape(H, 1)),
        "ident": np.eye(128, dtype=np.float32),
        "zeros": np.zeros((128, O), np.float32),
    }
    in_maps = []
    for c in range(NCORES):
        xc = x[c * BL:(c + 1) * BL, :].astype(np.float32)
        in_maps.append(dict(
            common,
            xT=_r32r(np.ascontiguousarray(xc.T)),
            xrevT=_r32r(np.ascontiguousarray(xc[:, ::-1].T)),
        ))
    return in_maps


def assemble(results):
    Y = np.empty((O, T, B), np.float32)
    for c in range(NCORES):
        yc = results[c]["y_out"]                      # [1024, 4096]
        yc = yc.reshape(NCH, BL, C, O)                # [j, b, i, o]
        Y[:, :, c * BL:(c + 1) * BL] = (
            yc.transpose(3, 0, 2, 1).reshape(O, T, BL))
    return Y


def kernel(x, W1, W2, W3):
    nc = _get_nc(False)
    in_maps = make_in_maps(np.asarray(x), np.asarray(W1),
                           np.asarray(W2), np.asarray(W3))
    res = run_bass_kernel_spmd(nc, in_maps, list(range(NCORES))).results
    return assemble(res)


if __name__ == "__main__":
    rng = np.random.default_rng(0)
    x = rng.standard_normal((B, T)).astype(np.float32)
    W1 = rng.standard_normal((H, 1)).astype(np.float32) * 0.001
    W2 = rng.standard_normal((H, H)).astype(np.float32) * 0.001
    W3 = rng.standard_normal((O, H)).astype(np.float32) * 0.001
    y = kernel(x, W1, W2, W3)
    print("ok", y.shape, np.isfinite(y).mean())
